# revision 1
# baseline (speedup 1.0000x reference)
"""Bass/Tile TP attention kernel for trn2, 8 NeuronCores.

Strategy (sequence-parallel attention):
  - core r owns query rows [512r, 512r+512)
  - RMS-norm own rows -> PE-transpose -> AllGather xq^T (fp16, E-major)
  - column-shard K^T/V projections (512 feature cols each) + RoPE, AllGather
  - Q^T for own rows via full w_q (no collective)
  - scores -> causal mask -> softmax -> P^T -> attn^T = V x P^T
  - out rows = attn^T^T @ w_out, host concatenates row blocks

All matmul layouts keep the contraction dim on partitions (lhsT convention).
"""

import numpy as np

S = 4096
E = 4096
NC = 8
RB = S // NC          # 512 rows per core
P = 128
KO = E // P           # 32 k-tiles
NCH = 8               # 512-wide chunks over S or E
CH = 512
EPS = 1e-6
BASE_THETA = 10000.0

_BUILT = None


def _build_nc():
    import concourse.bass as bass
    import concourse.mybir as mybir
    import concourse.tile as tile
    from concourse import bacc
    from concourse.masks import make_identity

    dt16 = mybir.dt.float16
    dt32 = mybir.dt.float32
    AX = mybir.AxisListType.X
    mult = mybir.AluOpType.mult
    addop = mybir.AluOpType.add
    maxop = mybir.AluOpType.max
    Copy = mybir.ActivationFunctionType.Copy
    Exp = mybir.ActivationFunctionType.Exp

    nc = bacc.Bacc(
        "TRN2", target_bir_lowering=False, debug=False, num_devices=NC)

    # I/O
    x_r = nc.dram_tensor("x_r", [RB, E], dt32, kind="ExternalInput")
    wq = nc.dram_tensor("wq", [E, E], dt32, kind="ExternalInput")
    wk_c = nc.dram_tensor("wk_c", [E, RB], dt32, kind="ExternalInput")
    wv_c = nc.dram_tensor("wv_c", [E, RB], dt32, kind="ExternalInput")
    wo = nc.dram_tensor("wo", [E, E], dt32, kind="ExternalInput")
    scale_d = nc.dram_tensor("scale", [KO, P], dt32, kind="ExternalInput")
    cos_k = nc.dram_tensor("cos_k", [RB, S], dt16, kind="ExternalInput")
    sin_k = nc.dram_tensor("sin_k", [RB, S], dt16, kind="ExternalInput")
    cos_q = nc.dram_tensor("cos_q", [E, RB], dt16, kind="ExternalInput")
    sin_q = nc.dram_tensor("sin_q", [E, RB], dt16, kind="ExternalInput")
    amask = nc.dram_tensor("amask", [RB, S], dt16, kind="ExternalInput")
    swapm_d = nc.dram_tensor("swapm", [P, P], dt16, kind="ExternalInput")
    out_r = nc.dram_tensor("out_r", [RB, E], dt16, kind="ExternalOutput")

    # internal DRAM (collective bounces); outputs Shared for HBM-HBM perf
    xqT_b = nc.dram_tensor("xqT_b", [E, RB], dt16)
    xqT_all = nc.dram_tensor("xqT_all", [NC * E, RB], dt16, addr_space="Shared")
    kT_b = nc.dram_tensor("kT_b", [RB, S], dt16)
    kT_all = nc.dram_tensor("kT_all", [E, S], dt16, addr_space="Shared")
    v_b = nc.dram_tensor("v_b", [S, RB], dt16)
    v_all = nc.dram_tensor("v_all", [NC * S, RB], dt16, addr_space="Shared")
    RG = [list(range(NC))]

    with tile.TileContext(nc) as tc:
        with tc.tile_pool(name="const", bufs=1) as constp, \
             tc.tile_pool(name="psB", bufs=4, space="PSUM") as psB, \
             tc.tile_pool(name="psA", bufs=1, space="PSUM") as psA, \
             tc.tile_pool(name="pstr", bufs=2, space="PSUM") as pstr, \
             tc.tile_pool(name="pssw", bufs=1, space="PSUM") as pssw:

            ident = constp.tile([P, P], dt16, tag="ident")
            make_identity(nc, ident)
            swap_sb = constp.tile([P, P], dt16, tag="swap")
            nc.sync.dma_start(swap_sb, swapm_d[:])
            scale_sb = constp.tile([P, KO], dt32, tag="scale")
            nc.sync.dma_start(scale_sb, scale_d[:].rearrange("ko p -> p ko"))

            with tc.tile_pool(name="qTp", bufs=1) as qTp:
                qT = qTp.tile([P, KO, RB], dt16, tag="qT")

                with tc.tile_pool(name="xqTrp", bufs=1) as xqTrp:
                    xqT_r = xqTrp.tile([P, KO, RB], dt16, tag="xqT_r")

                    # ---- stage A: RMS norm of own rows + transpose ----
                    with tc.tile_pool(name="normp", bufs=2) as normp, \
                         tc.tile_pool(name="nstat", bufs=2) as nstat:
                        for t in range(RB // P):
                            x_sb = normp.tile([P, E], dt32, tag="x")
                            nc.sync.dma_start(x_sb, x_r[t * P:(t + 1) * P, :])
                            sq = normp.tile([P, E], dt32, tag="sq")
                            ssum = nstat.tile([P, 1], dt32, tag="ssum")
                            nc.scalar.activation(
                                sq, x_sb, mybir.ActivationFunctionType.Square,
                                accum_out=ssum)
                            s2 = nstat.tile([P, 1], dt32, tag="s2")
                            nc.vector.tensor_scalar(s2, ssum, 1.0 / E, EPS, mult, addop)
                            s3 = nstat.tile([P, 1], dt32, tag="s3")
                            nc.scalar.sqrt(s3, s2)
                            rinv = nstat.tile([P, 1], dt32, tag="rinv")
                            nc.vector.reciprocal(rinv, s3)
                            xq_sb = normp.tile([P, E], dt16, tag="xq")
                            nc.scalar.activation(xq_sb, x_sb, Copy, scale=rinv[:, 0:1])
                            for c in range(KO):
                                pt = pstr.tile([P, P], dt16, tag="tr")
                                nc.tensor.transpose(pt, xq_sb[:, c * P:(c + 1) * P], ident)
                                nc.scalar.copy(xqT_r[:, c, t * P:(t + 1) * P], pt)

                    nc.sync.dma_start(
                        xqT_b[:].rearrange("(ko p) s -> p ko s", p=P), xqT_r[:])
                    nc.gpsimd.collective_compute(
                        "AllGather", mybir.AluOpType.bypass, replica_groups=RG,
                        ins=[xqT_b[:]], outs=[xqT_all[:]])

                    # ---- stage QT: qT = wq^T @ xq^T for own rows (+rope) ----
                    with tc.tile_pool(name="wqs", bufs=6) as wqs, \
                         tc.tile_pool(name="wqs16", bufs=6) as wqs16, \
                         tc.tile_pool(name="ropes", bufs=2) as ropes:
                        for mg in range(8):
                            pss = [psB.tile([P, CH], dt32, tag="mm4", name=f"ps{i}") for i in range(4)]
                            for k in range(KO):
                                wq32 = wqs.tile([P, CH], dt32, tag="wq32")
                                nc.sync.dma_start(
                                    wq32, wq[k * P:(k + 1) * P, mg * CH:(mg + 1) * CH])
                                wq16 = wqs16.tile([P, CH], dt16, tag="wq16")
                                nc.vector.tensor_scalar_mul(wq16, wq32, scale_sb[:, k:k + 1])
                                for m4 in range(4):
                                    nc.tensor.matmul(
                                        pss[m4], lhsT=wq16[:, m4 * P:(m4 + 1) * P],
                                        rhs=xqT_r[:, k, :],
                                        start=(k == 0), stop=(k == KO - 1))
                            for m4 in range(4):
                                m = mg * 4 + m4
                                nc.scalar.copy(qT[:, m, :], pss[m4])
                                sw = pssw.tile([P, CH], dt32, tag="sw")
                                nc.tensor.matmul(sw, lhsT=swap_sb, rhs=qT[:, m, :],
                                                 start=True, stop=True)
                                cq = ropes.tile([P, CH], dt16, tag="cq")
                                nc.sync.dma_start(cq, cos_q[m * P:(m + 1) * P, :])
                                sq_ = ropes.tile([P, CH], dt16, tag="sq")
                                nc.sync.dma_start(sq_, sin_q[m * P:(m + 1) * P, :])
                                t1 = ropes.tile([P, CH], dt16, tag="t1")
                                nc.vector.tensor_tensor(t1, qT[:, m, :], cq, mult)
                                t2 = ropes.tile([P, CH], dt16, tag="t2")
                                nc.vector.tensor_tensor(t2, sw, sq_, mult)
                                nc.vector.tensor_tensor(qT[:, m, :], t1, t2, addop)

                    # ---- stage KV: kT_c (+rope) and V_c; AllGathers ----
                    xqa = xqT_all[:].rearrange("(c ko p) s -> c p ko s", c=NC, p=P)
                    with tc.tile_pool(name="wkp", bufs=1) as wkp, \
                         tc.tile_pool(name="kvs", bufs=2) as kvs, \
                         tc.tile_pool(name="xqs", bufs=1) as xqs, \
                         tc.tile_pool(name="kropes", bufs=2) as kropes:
                        wk16 = wkp.tile([P, KO, RB], dt16, tag="wk16")
                        wv16 = wkp.tile([P, KO, RB], dt16, tag="wv16")
                        for k in range(KO):
                            w32a = kvs.tile([P, RB], dt32, tag="wkv32")
                            nc.sync.dma_start(w32a, wk_c[k * P:(k + 1) * P, :])
                            nc.vector.tensor_scalar_mul(
                                wk16[:, k, :], w32a, scale_sb[:, k:k + 1])
                            w32b = kvs.tile([P, RB], dt32, tag="wkv32")
                            nc.sync.dma_start(w32b, wv_c[k * P:(k + 1) * P, :])
                            nc.vector.tensor_scalar_mul(
                                wv16[:, k, :], w32b, scale_sb[:, k:k + 1])

                        for n in range(NCH):
                            xq_q = []
                            for qq in range(4):
                                xt = xqs.tile([P, 8, CH], dt16, tag=f"xqch{qq}")
                                nc.sync.dma_start(xt, xqa[n, :, qq * 8:(qq + 1) * 8, :])
                                xq_q.append(xt)
                            # K^T chunk: 4 eo-tiles x this s-chunk
                            pss = [psB.tile([P, CH], dt32, tag="mm4", name=f"ps{i}") for i in range(4)]
                            for k in range(KO):
                                rhs = xq_q[k // 8][:, k % 8, :]
                                for m4 in range(4):
                                    nc.tensor.matmul(
                                        pss[m4], lhsT=wk16[:, k, m4 * P:(m4 + 1) * P],
                                        rhs=rhs, start=(k == 0), stop=(k == KO - 1))
                            for m4 in range(4):
                                kev = kvs.tile([P, CH], dt16, tag="kev")
                                nc.scalar.copy(kev, pss[m4])
                                sw = pssw.tile([P, CH], dt32, tag="sw")
                                nc.tensor.matmul(sw, lhsT=swap_sb, rhs=kev,
                                                 start=True, stop=True)
                                ck = kropes.tile([P, CH], dt16, tag="ck")
                                nc.sync.dma_start(
                                    ck, cos_k[m4 * P:(m4 + 1) * P, n * CH:(n + 1) * CH])
                                sk = kropes.tile([P, CH], dt16, tag="sk")
                                nc.sync.dma_start(
                                    sk, sin_k[m4 * P:(m4 + 1) * P, n * CH:(n + 1) * CH])
                                t1 = kropes.tile([P, CH], dt16, tag="kt1")
                                nc.vector.tensor_tensor(t1, kev, ck, mult)
                                t2 = kropes.tile([P, CH], dt16, tag="kt2")
                                nc.vector.tensor_tensor(t2, sw, sk, mult)
                                kro = kvs.tile([P, CH], dt16, tag="kro")
                                nc.vector.tensor_tensor(kro, t1, t2, addop)
                                for jj in range(4):
                                    g = 8 * jj + n  # global 128-col block
                                    nc.sync.dma_start(
                                        kT_b[m4 * P:(m4 + 1) * P, g * P:(g + 1) * P],
                                        kro[:, jj * P:(jj + 1) * P])
                            # V natural for the same xq chunk
                            for ms in range(4):
                                psv = psA.tile([P, CH], dt32, tag="mm2")
                                for k in range(KO):
                                    nc.tensor.matmul(
                                        psv,
                                        lhsT=xq_q[k // 8][:, k % 8, ms * P:(ms + 1) * P],
                                        rhs=wv16[:, k, :],
                                        start=(k == 0), stop=(k == KO - 1))
                                vev = kvs.tile([P, CH], dt16, tag="vev")
                                nc.scalar.copy(vev, psv)
                                st = 8 * ms + n  # global 128-row block
                                nc.sync.dma_start(v_b[st * P:(st + 1) * P, :], vev)

                        nc.gpsimd.collective_compute(
                            "AllGather", mybir.AluOpType.bypass, replica_groups=RG,
                            ins=[kT_b[:]], outs=[kT_all[:]])
                        nc.gpsimd.collective_compute(
                            "AllGather", mybir.AluOpType.bypass, replica_groups=RG,
                            ins=[v_b[:]], outs=[v_all[:]])

                # xqT_r freed here
                with tc.tile_pool(name="attnTp", bufs=1) as attnTp:
                  attnT = attnTp.tile([P, KO, RB], dt16, tag="attnT")
                  with tc.tile_pool(name="PTp", bufs=1) as PTp:
                    PT = PTp.tile([P, KO, RB], dt16, tag="PT")

                    # ---- scores + softmax + P^T ----
                    with tc.tile_pool(name="Pp", bufs=1) as Pp, \
                         tc.tile_pool(name="kts", bufs=1) as kts, \
                         tc.tile_pool(name="sstat", bufs=4) as sstat, \
                         tc.tile_pool(name="ams", bufs=4) as ams:
                        Ptile = Pp.tile([P, 4, S], dt16, tag="P")
                        kta = kT_all[:].rearrange("(ko p) s -> p ko s", p=P)
                        for n in range(NCH):
                            # query-tile j only needs key chunks 0..2j+1 (causal)
                            allowed = [m4 for m4 in range(4) if n <= 2 * m4 + 1]
                            kt_q = []
                            for qq in range(4):
                                kt = kts.tile([P, 8, CH], dt16, tag=f"ktch{qq}")
                                nc.sync.dma_start(
                                    kt, kta[:, qq * 8:(qq + 1) * 8, n * CH:(n + 1) * CH])
                                kt_q.append(kt)
                            pss = {m4: psB.tile([P, CH], dt32, tag="mm4", name=f"ps{m4}")
                                   for m4 in allowed}
                            for k in range(KO):
                                rhs = kt_q[k // 8][:, k % 8, :]
                                for m4 in allowed:
                                    nc.tensor.matmul(
                                        pss[m4], lhsT=qT[:, k, m4 * P:(m4 + 1) * P],
                                        rhs=rhs, start=(k == 0), stop=(k == KO - 1))
                            for m4 in allowed:
                                am = ams.tile([P, CH], dt16, tag="am")
                                nc.sync.dma_start(
                                    am, amask[m4 * P:(m4 + 1) * P, n * CH:(n + 1) * CH])
                                nc.vector.scalar_tensor_tensor(
                                    out=Ptile[:, m4, n * CH:(n + 1) * CH],
                                    in0=pss[m4], scalar=1.0 / 64.0, in1=am,
                                    op0=mult, op1=addop)
                        for m in range(4):
                            L = (2 * m + 2) * CH  # causal prefix length
                            negmax = sstat.tile([P, 1], dt32, tag="nm")
                            nc.vector.tensor_reduce(
                                negmax, Ptile[:, m, :L], axis=AX, op=maxop, negate=True)
                            ssum = sstat.tile([P, 1], dt32, tag="sm")
                            nc.scalar.activation(
                                Ptile[:, m, :L], Ptile[:, m, :L], Exp,
                                bias=negmax[:, 0:1], scale=1.0, accum_out=ssum)
                            rinv = sstat.tile([P, 1], dt32, tag="ri")
                            nc.vector.reciprocal(rinv, ssum)
                            nc.scalar.mul(Ptile[:, m, :L], Ptile[:, m, :L], rinv[:, 0:1])
                            for st in range(8 * (m + 1)):
                                pt = pstr.tile([P, P], dt16, tag="tr")
                                nc.tensor.transpose(
                                    pt, Ptile[:, m, st * P:(st + 1) * P], ident)
                                nc.scalar.copy(PT[:, st, m * P:(m + 1) * P], pt)

                    # ---- attn^T = V x P^T ----
                    with tc.tile_pool(name="vs", bufs=2) as vs:
                        va = v_all[:].rearrange("(c ko p) eo -> c p ko eo", c=NC, p=P)
                        for m in range(KO):
                            c, sub = m // 4, m % 4
                            vt = vs.tile([P, KO, P], dt16, tag="vt")
                            nc.sync.dma_start(
                                vt, va[c, :, :, sub * P:(sub + 1) * P])
                            ps = psA.tile([P, CH], dt32, tag="mm2")
                            for k in range(KO):
                                j0 = k // 8  # query tiles j >= j0 attend key tile k
                                nc.tensor.matmul(
                                    ps[:, j0 * P:], lhsT=vt[:, k, :],
                                    rhs=PT[:, k, j0 * P:],
                                    start=(k == 0), stop=(k == KO - 1))
                            nc.scalar.copy(attnT[:, m, :], ps)

                  # ---- out = attn @ wo (rows stay ours) ----
                  with tc.tile_pool(name="wos", bufs=6) as wos, \
                       tc.tile_pool(name="wos16", bufs=6) as wos16, \
                       tc.tile_pool(name="oev", bufs=4) as oev:
                      for n in range(NCH):
                          pss = [psB.tile([P, CH], dt32, tag="mm4", name=f"ps{i}") for i in range(4)]
                          for k in range(KO):
                              w32 = wos.tile([P, CH], dt32, tag="wo32")
                              nc.sync.dma_start(
                                  w32, wo[k * P:(k + 1) * P, n * CH:(n + 1) * CH])
                              w16 = wos16.tile([P, CH], dt16, tag="wo16")
                              nc.vector.tensor_copy(w16, w32)
                              for mq in range(4):
                                  nc.tensor.matmul(
                                      pss[mq], lhsT=attnT[:, k, mq * P:(mq + 1) * P],
                                      rhs=w16, start=(k == 0), stop=(k == KO - 1))
                          for mq in range(4):
                              ot = oev.tile([P, CH], dt16, tag="ot")
                              nc.scalar.copy(ot, pss[mq])
                              nc.sync.dma_start(
                                  out_r[mq * P:(mq + 1) * P, n * CH:(n + 1) * CH], ot)

    nc.compile()
    return nc


def _tables():
    pos = np.arange(S, dtype=np.float32)[:, None]
    j = np.arange(E // 2, dtype=np.float32)[None, :]
    theta = pos / np.power(np.float32(BASE_THETA), 2.0 * j / np.float32(E))
    cos_t = np.cos(theta).astype(np.float16)   # (S, E/2)
    sin_t = np.sin(theta).astype(np.float16)
    cosE = np.repeat(cos_t, 2, axis=1).T.copy()            # (E, S)
    sgn = np.where(np.arange(E) % 2 == 0, np.float16(1), np.float16(-1))
    sinE = (np.repeat(sin_t, 2, axis=1) * sgn[None, :]).T.copy()  # (E, S)
    return cosE, sinE


def _own_rows(r):
    # core r owns 128-row blocks {8j + r : j=0..3}
    return np.concatenate(
        [np.arange(128 * (8 * j + r), 128 * (8 * j + r) + 128) for j in range(4)])


def _prep_in_maps(inputs):
    x = np.ascontiguousarray(np.asarray(inputs["x"], dtype=np.float32))
    w_q = np.ascontiguousarray(np.asarray(inputs["w_q"], dtype=np.float32))
    w_k = np.ascontiguousarray(np.asarray(inputs["w_k"], dtype=np.float32))
    w_v = np.ascontiguousarray(np.asarray(inputs["w_v"], dtype=np.float32))
    w_out = np.ascontiguousarray(np.asarray(inputs["w_out"], dtype=np.float32))
    sf = np.asarray(inputs["scaling_factor"], dtype=np.float32)

    cosE, sinE = _tables()
    swapm = np.zeros((P, P), dtype=np.float16)
    ii = np.arange(0, P, 2)
    swapm[ii, ii + 1] = np.float16(1)
    swapm[ii + 1, ii] = np.float16(1)
    scale_in = np.ascontiguousarray(sf.reshape(KO, P))

    col = np.arange(S)[None, :]
    # xqT AllGather chunk n holds core n's scattered rows; K-rope tables must
    # follow that column order
    perm = np.concatenate([_own_rows(n) for n in range(NC)])
    in_maps = []
    for r in range(NC):
        rows = _own_rows(r)
        row = rows[:, None]
        am = np.where(col > row, np.float16(-np.inf), np.float16(0.0)).astype(np.float16)
        in_maps.append({
            "x_r": np.ascontiguousarray(x[rows, :]),
            "wq": w_q,
            "wk_c": np.ascontiguousarray(w_k[:, r * RB:(r + 1) * RB]),
            "wv_c": np.ascontiguousarray(w_v[:, r * RB:(r + 1) * RB]),
            "wo": w_out,
            "scale": scale_in,
            "cos_k": np.ascontiguousarray(cosE[r * RB:(r + 1) * RB][:, perm]),
            "sin_k": np.ascontiguousarray(sinE[r * RB:(r + 1) * RB][:, perm]),
            "cos_q": np.ascontiguousarray(cosE[:, rows]),
            "sin_q": np.ascontiguousarray(sinE[:, rows]),
            "amask": am,
            "swapm": swapm,
        })
    return in_maps


def _run(inputs, trace=False, **kw):
    global _BUILT
    from concourse.bass_utils import run_bass_kernel_spmd
    if _BUILT is None:
        _BUILT = _build_nc()
    in_maps = _prep_in_maps(inputs)
    res = run_bass_kernel_spmd(_BUILT, in_maps, list(range(NC)), trace=trace, **kw)
    out = np.empty((S, E), dtype=np.float16)
    for r in range(NC):
        out[_own_rows(r)] = np.asarray(res.results[r]["out_r"]).astype(np.float16)
    return out, res


def kernel(**inputs):
    out, _ = _run(inputs, trace=False)
    return out



# revision 6
# speedup vs baseline: 1.1431x; 1.1431x over previous
"""Bass/Tile TP attention kernel for trn2, 8 NeuronCores.

Sequence-parallel attention, restructured for a gapless tensor-engine
stream (TRN2 PE p-state needs ~3us of continuous busy to reach 2.4GHz):

  - weights pre-scaled by scaling_factor and cast to fp16 on host
  - wq/wk columns host-permuted even/odd per 512-block so RoPE pairs sit
    in separate 128-row tiles -> rope is pure element-wise DVE work
    (no PE swap matmuls, no scalar-copy -> PE dependency)
  - phase order: norm -> [xq AllGather || Q proj] -> K proj (global col
    order) -> [kT AllGather || V proj] -> [v AllGather || scores+softmax]
    -> attnV -> out proj
  - softmax: 1/64 folded into exp scale, ln64 shift keeps unnormalized
    P/attn in fp16 range, 1/sum applied free at out-proj psum eviction
  - single-psum-bank accumulation chains (32 matmuls each), 4-deep bank
    rotation so evictions overlap the next chains

core r owns query row blocks {128*(8j+r) : j=0..3} (causal balance).
"""

import numpy as np

S = 4096
E = 4096
NC = 8
RB = S // NC          # 512 rows per core
P = 128
KO = E // P           # 32 k-tiles
NCH = 8               # 512-wide chunks over S or E
CH = 512
HALF = E // 2
EPS = 1e-6
BASE_THETA = 10000.0
LOG64 = float(np.log(64.0))

_BUILT = None


def _build_nc():
    import concourse.bass as bass
    import concourse.mybir as mybir
    import concourse.tile as tile
    from concourse import bacc
    from concourse.masks import make_identity

    dt16 = mybir.dt.float16
    dt32 = mybir.dt.float32
    AX = mybir.AxisListType.X
    mult = mybir.AluOpType.mult
    addop = mybir.AluOpType.add
    subop = mybir.AluOpType.subtract
    maxop = mybir.AluOpType.max
    Copy = mybir.ActivationFunctionType.Copy
    Exp = mybir.ActivationFunctionType.Exp
    Square = mybir.ActivationFunctionType.Square

    nc = bacc.Bacc(
        "TRN2", target_bir_lowering=False, debug=False, num_devices=NC)

    # I/O (weights arrive fp16, pre-scaled, wq/wk col-permuted even/odd)
    x_r = nc.dram_tensor("x_r", [RB, E], dt32, kind="ExternalInput")
    wq = nc.dram_tensor("wq", [E, E], dt16, kind="ExternalInput")
    wk_c = nc.dram_tensor("wk_c", [E, RB], dt16, kind="ExternalInput")
    wv_c = nc.dram_tensor("wv_c", [E, RB], dt16, kind="ExternalInput")
    wo = nc.dram_tensor("wo", [E, E], dt16, kind="ExternalInput")
    cosq_d = nc.dram_tensor("cosq", [HALF, RB], dt16, kind="ExternalInput")
    sinq_d = nc.dram_tensor("sinq", [HALF, RB], dt16, kind="ExternalInput")
    cosk_d = nc.dram_tensor("cosk", [2 * P, S], dt16, kind="ExternalInput")
    sink_d = nc.dram_tensor("sink", [2 * P, S], dt16, kind="ExternalInput")
    amask = nc.dram_tensor("amask", [RB, 2 * CH], dt16, kind="ExternalInput")
    out_r = nc.dram_tensor("out_r", [RB, E], dt16, kind="ExternalOutput")

    # internal DRAM (collective bounces); outputs Shared for HBM-HBM perf
    xqT_b = nc.dram_tensor("xqT_b", [E, RB], dt16)
    xqT_all = nc.dram_tensor("xqT_all", [NC * E, RB], dt16, addr_space="Shared")
    kT_b = nc.dram_tensor("kT_b", [RB, S], dt16)
    kT_all = nc.dram_tensor("kT_all", [E, S], dt16, addr_space="Shared")
    v_b = nc.dram_tensor("v_b", [S, RB], dt16)
    v_all = nc.dram_tensor("v_all", [NC * S, RB], dt16, addr_space="Shared")
    RG = [list(range(NC))]

    with tile.TileContext(nc) as tc:
        with tc.tile_pool(name="const", bufs=1) as constp, \
             tc.tile_pool(name="psQ", bufs=4, space="PSUM") as psQ, \
             tc.tile_pool(name="pstr", bufs=4, space="PSUM") as pstr:

            ident = constp.tile([P, P], dt16, tag="ident")
            make_identity(nc, ident)
            rinv_all = constp.tile([P, 4], dt32, tag="rinv_all")

            with tc.tile_pool(name="qTp", bufs=1) as qTp:
                qT = qTp.tile([P, KO, RB], dt16, tag="qT")

                with tc.tile_pool(name="xqTrp", bufs=1) as xqTrp:
                    xqT_r = xqTrp.tile([P, KO, RB], dt16, tag="xqT_r")

                    # ---- stage A: RMS norm of own rows + transpose ----
                    with tc.tile_pool(name="normp", bufs=2) as normp, \
                         tc.tile_pool(name="nstat", bufs=2) as nstat:
                        for t in range(RB // P):
                            x_sb = normp.tile([P, E], dt32, tag="x")
                            nc.sync.dma_start(x_sb, x_r[t * P:(t + 1) * P, :])
                            sq = normp.tile([P, E], dt32, tag="sq")
                            ssum = nstat.tile([P, 1], dt32, tag="ssum")
                            nc.scalar.activation(sq, x_sb, Square, accum_out=ssum)
                            s2 = nstat.tile([P, 1], dt32, tag="s2")
                            nc.vector.tensor_scalar(s2, ssum, 1.0 / E, EPS, mult, addop)
                            s3 = nstat.tile([P, 1], dt32, tag="s3")
                            nc.scalar.sqrt(s3, s2)
                            rinv = nstat.tile([P, 1], dt32, tag="rinv")
                            nc.vector.reciprocal(rinv, s3)
                            xq_sb = normp.tile([P, E], dt16, tag="xq")
                            nc.vector.tensor_scalar_mul(xq_sb, x_sb, rinv[:, 0:1])
                            for c in range(KO):
                                pt = pstr.tile([P, P], dt16, tag="tr")
                                nc.tensor.transpose(pt, xq_sb[:, c * P:(c + 1) * P], ident)
                                nc.scalar.copy(xqT_r[:, c, t * P:(t + 1) * P], pt)
                            # ship this row-tile of xq^T out as soon as ready
                            nc.sync.dma_start(
                                xqT_b[:, t * P:(t + 1) * P].rearrange(
                                    "(ko p) s -> p ko s", p=P),
                                xqT_r[:, :, t * P:(t + 1) * P])

                    nc.gpsimd.collective_compute(
                        "AllGather", mybir.AluOpType.bypass, replica_groups=RG,
                        ins=[xqT_b[:]], outs=[xqT_all[:]])

                    # ---- stage QT: qT = wq^T @ xq^T for own rows, + rope ----
                    with tc.tile_pool(name="wqs", bufs=2) as wqs, \
                         tc.tile_pool(name="qropes", bufs=2) as qropes, \
                         tc.tile_pool(name="qrtmp", bufs=2) as qrtmp:
                        for mg in range(8):
                            wqt = wqs.tile([P, KO, CH], dt16, tag="wqt")
                            nc.sync.dma_start(
                                wqt, wq[:, mg * CH:(mg + 1) * CH].rearrange(
                                    "(ko p) m -> p ko m", p=P))
                            for m4 in range(4):
                                m = mg * 4 + m4
                                ps = psQ.tile([P, CH], dt32, tag="mm")
                                for k in range(KO):
                                    nc.tensor.matmul(
                                        ps, lhsT=wqt[:, k, m4 * P:(m4 + 1) * P],
                                        rhs=xqT_r[:, k, :],
                                        start=(k == 0), stop=(k == KO - 1))
                                nc.scalar.copy(qT[:, m, :], ps)
                            # rope this 512-col block: pairs (4mg+h, 4mg+2+h)
                            for h in range(2):
                                me, mo = mg * 4 + h, mg * 4 + 2 + h
                                j0 = mg * 2 + h  # 128-row block into cosq/sinq
                                cq = qropes.tile([P, RB], dt16, tag="cq")
                                nc.sync.dma_start(cq, cosq_d[j0 * P:(j0 + 1) * P, :])
                                sq_ = qropes.tile([P, RB], dt16, tag="sq")
                                nc.sync.dma_start(sq_, sinq_d[j0 * P:(j0 + 1) * P, :])
                                t1 = qrtmp.tile([P, RB], dt16, tag="t1")
                                nc.vector.tensor_tensor(t1, qT[:, me, :], cq, mult)
                                t3 = qrtmp.tile([P, RB], dt16, tag="t3")
                                nc.vector.tensor_tensor(t3, qT[:, me, :], sq_, mult)
                                t4 = qrtmp.tile([P, RB], dt16, tag="t4")
                                nc.vector.tensor_tensor(t4, qT[:, mo, :], sq_, mult)
                                nc.vector.tensor_tensor(qT[:, me, :], t1, t4, addop)
                                t5 = qrtmp.tile([P, RB], dt16, tag="t5")
                                nc.vector.tensor_tensor(t5, qT[:, mo, :], cq, mult)
                                nc.vector.tensor_tensor(qT[:, mo, :], t5, t3, subop)

                # xqT_r freed here
                xqa = xqT_all[:].rearrange("(c ko p) s -> c p ko s", c=NC, p=P)
                with tc.tile_pool(name="wkvp", bufs=1) as wkvp, \
                     tc.tile_pool(name="xqs", bufs=2) as xqs, \
                     tc.tile_pool(name="kevs", bufs=8) as kevs, \
                     tc.tile_pool(name="ktabs", bufs=2) as ktabs, \
                     tc.tile_pool(name="krtmp", bufs=2) as krtmp:
                    wk16 = wkvp.tile([P, KO, RB], dt16, tag="wk16")
                    nc.sync.dma_start(
                        wk16, wk_c[:].rearrange("(ko p) m -> p ko m", p=P))
                    wv16 = wkvp.tile([P, KO, RB], dt16, tag="wv16")
                    nc.sync.dma_start(
                        wv16, wv_c[:].rearrange("(ko p) m -> p ko m", p=P))

                    # ---- stage K: kT for own feature shard, + rope ----
                    # chunk n of the gather holds core n's scattered rows;
                    # kT_b is written in GLOBAL column order via per-128
                    # scatter (global block g = 8*jj + n)
                    for n in range(NCH):
                        xqg = xqs.tile([P, KO, CH], dt16, tag="xqg")
                        nc.sync.dma_start(xqg, xqa[n, :, :, :])
                        kev = [kevs.tile([P, CH], dt16, tag="kev", name=f"kev{i}")
                               for i in range(4)]
                        for m4 in range(4):
                            ps = psQ.tile([P, CH], dt32, tag="mm")
                            for k in range(KO):
                                nc.tensor.matmul(
                                    ps, lhsT=wk16[:, k, m4 * P:(m4 + 1) * P],
                                    rhs=xqg[:, k, :],
                                    start=(k == 0), stop=(k == KO - 1))
                            nc.scalar.copy(kev[m4], ps)
                        for h in range(2):
                            ck = ktabs.tile([P, CH], dt16, tag="ck")
                            nc.sync.dma_start(
                                ck, cosk_d[h * P:(h + 1) * P, n * CH:(n + 1) * CH])
                            sk = ktabs.tile([P, CH], dt16, tag="sk")
                            nc.sync.dma_start(
                                sk, sink_d[h * P:(h + 1) * P, n * CH:(n + 1) * CH])
                            t1 = krtmp.tile([P, CH], dt16, tag="t1")
                            nc.vector.tensor_tensor(t1, kev[h], ck, mult)
                            t3 = krtmp.tile([P, CH], dt16, tag="t3")
                            nc.vector.tensor_tensor(t3, kev[h], sk, mult)
                            t4 = krtmp.tile([P, CH], dt16, tag="t4")
                            nc.vector.tensor_tensor(t4, kev[2 + h], sk, mult)
                            kroE = kevs.tile([P, CH], dt16, tag="kro", name="kroE")
                            nc.vector.tensor_tensor(kroE, t1, t4, addop)
                            t5 = krtmp.tile([P, CH], dt16, tag="t5")
                            nc.vector.tensor_tensor(t5, kev[2 + h], ck, mult)
                            kroO = kevs.tile([P, CH], dt16, tag="kro", name="kroO")
                            nc.vector.tensor_tensor(kroO, t5, t3, subop)
                            for jj in range(4):
                                g = 8 * jj + n  # global 128-col block
                                nc.sync.dma_start(
                                    kT_b[h * P:(h + 1) * P, g * P:(g + 1) * P],
                                    kroE[:, jj * P:(jj + 1) * P])
                                nc.sync.dma_start(
                                    kT_b[(2 + h) * P:(3 + h) * P,
                                         g * P:(g + 1) * P],
                                    kroO[:, jj * P:(jj + 1) * P])

                    nc.gpsimd.collective_compute(
                        "AllGather", mybir.AluOpType.bypass, replica_groups=RG,
                        ins=[kT_b[:]], outs=[kT_all[:]])

                    # ---- stage V: v rows in GLOBAL order ----
                    for n in range(NCH):
                        xqg = xqs.tile([P, KO, CH], dt16, tag="xqg")
                        nc.sync.dma_start(xqg, xqa[n, :, :, :])
                        for ms in range(4):
                            ps = psQ.tile([P, CH], dt32, tag="mm")
                            for k in range(KO):
                                nc.tensor.matmul(
                                    ps, lhsT=xqg[:, k, ms * P:(ms + 1) * P],
                                    rhs=wv16[:, k, :],
                                    start=(k == 0), stop=(k == KO - 1))
                            vev = kevs.tile([P, CH], dt16, tag="vev")
                            nc.scalar.copy(vev, ps)
                            st = 8 * ms + n  # global 128-row block
                            nc.sync.dma_start(
                                v_b[st * P:(st + 1) * P, :], vev)

                    nc.gpsimd.collective_compute(
                        "AllGather", mybir.AluOpType.bypass, replica_groups=RG,
                        ins=[v_b[:]], outs=[v_all[:]])

                with tc.tile_pool(name="PTp", bufs=1) as PTp:
                    PT = PTp.tile([P, KO, RB], dt16, tag="PT")

                    # ---- scores + softmax + P^T ----
                    kta = kT_all[:].rearrange("(ko p) s -> p ko s", p=P)
                    with tc.tile_pool(name="Pp", bufs=1) as Pp, \
                         tc.tile_pool(name="kts", bufs=2) as kts, \
                         tc.tile_pool(name="sstat", bufs=4) as sstat, \
                         tc.tile_pool(name="ams", bufs=2) as ams:
                        Ptile = Pp.tile([P, 4, S], dt16, tag="P")

                        def softmax(m):
                            L = (2 * m + 2) * CH  # causal prefix length
                            maxv = sstat.tile([P, 1], dt32, tag="mx")
                            nc.vector.tensor_reduce(
                                maxv, Ptile[:, m, :L], axis=AX, op=maxop)
                            negb = sstat.tile([P, 1], dt32, tag="nb")
                            nc.vector.tensor_scalar(
                                negb, maxv, -1.0 / 64.0, -LOG64, mult, addop)
                            ssum = sstat.tile([P, 1], dt32, tag="sm")
                            nc.scalar.activation(
                                Ptile[:, m, :L], Ptile[:, m, :L], Exp,
                                bias=negb[:, 0:1], scale=1.0 / 64.0,
                                accum_out=ssum)
                            nc.vector.reciprocal(rinv_all[:, m:m + 1], ssum)

                        def p_transpose(m):
                            for st in range(8 * (m + 1)):
                                pt = pstr.tile([P, P], dt16, tag="tr")
                                nc.tensor.transpose(
                                    pt, Ptile[:, m, st * P:(st + 1) * P], ident)
                                nc.scalar.copy(PT[:, st, m * P:(m + 1) * P], pt)

                        for n in range(NCH):
                            allowed = [m4 for m4 in range(4) if n <= 2 * m4 + 1]
                            kt = kts.tile([P, KO, CH], dt16, tag="kt")
                            nc.sync.dma_start(kt, kta[:, :, n * CH:(n + 1) * CH])
                            for m4 in allowed:
                                ps = psQ.tile([P, CH], dt32, tag="mm")
                                for k in range(KO):
                                    nc.tensor.matmul(
                                        ps, lhsT=qT[:, k, m4 * P:(m4 + 1) * P],
                                        rhs=kt[:, k, :],
                                        start=(k == 0), stop=(k == KO - 1))
                                if n in (2 * m4, 2 * m4 + 1):
                                    am = ams.tile([P, CH], dt16, tag="am")
                                    nc.sync.dma_start(
                                        am, amask[m4 * P:(m4 + 1) * P,
                                                  (n - 2 * m4) * CH:
                                                  (n - 2 * m4 + 1) * CH])
                                    nc.vector.tensor_tensor(
                                        Ptile[:, m4, n * CH:(n + 1) * CH],
                                        ps, am, addop)
                                else:
                                    nc.scalar.copy(
                                        Ptile[:, m4, n * CH:(n + 1) * CH], ps)
                            # m4 finishes its prefix at n == 2*m4+1
                            if n == 1:
                                softmax(0)
                                p_transpose(0)
                            elif n == 3:
                                softmax(1)
                                p_transpose(1)
                            elif n == 5:
                                softmax(2)
                            elif n == 6:
                                p_transpose(2)
                            elif n == 7:
                                softmax(3)
                                p_transpose(3)

                    with tc.tile_pool(name="attnTp", bufs=1) as attnTp:
                        attnT = attnTp.tile([P, KO, RB], dt16, tag="attnT")

                        # ---- attn^T = V x P^T ----
                        va = v_all[:].rearrange(
                            "(c ko p) eo -> c p ko eo", c=NC, p=P)
                        with tc.tile_pool(name="vs", bufs=4) as vs:
                            for m in range(KO):
                                c, sub = m // 4, m % 4
                                vt = vs.tile([P, KO, P], dt16, tag="vt")
                                nc.sync.dma_start(
                                    vt, va[c, :, :, sub * P:(sub + 1) * P])
                                ps = psQ.tile([P, CH], dt32, tag="mm")
                                for k in range(KO):
                                    j0 = k // 8
                                    nc.tensor.matmul(
                                        ps[:, j0 * P:], lhsT=vt[:, k, :],
                                        rhs=PT[:, k, j0 * P:],
                                        start=(k == 0), stop=(k == KO - 1))
                                nc.scalar.copy(attnT[:, m, :], ps)

                        # ---- out = attn @ wo, rows stay ours; /sum here ----
                        with tc.tile_pool(name="wos", bufs=2) as wos, \
                             tc.tile_pool(name="oev", bufs=4) as oev:
                            for n in range(NCH):
                                wot = wos.tile([P, KO, CH], dt16, tag="wot")
                                nc.sync.dma_start(
                                    wot, wo[:, n * CH:(n + 1) * CH].rearrange(
                                        "(ko p) m -> p ko m", p=P))
                                for mq in range(4):
                                    ps = psQ.tile([P, CH], dt32, tag="mm")
                                    for k in range(KO):
                                        nc.tensor.matmul(
                                            ps,
                                            lhsT=attnT[:, k, mq * P:(mq + 1) * P],
                                            rhs=wot[:, k, :],
                                            start=(k == 0), stop=(k == KO - 1))
                                    ot = oev.tile([P, CH], dt16, tag="ot")
                                    nc.scalar.activation(
                                        ot, ps, Copy,
                                        scale=rinv_all[:, mq:mq + 1])
                                    nc.sync.dma_start(
                                        out_r[mq * P:(mq + 1) * P,
                                              n * CH:(n + 1) * CH], ot)

    nc.compile()
    return nc


def _feature_perm():
    # per 512-block: first 256 even global features, then 256 odd
    blocks = []
    for c in range(NC):
        j = np.arange(256) + c * 256
        blocks.append(2 * j)
        blocks.append(2 * j + 1)
    return np.concatenate(blocks)


def _rope_tables():
    # feature-major half tables: theta[j, s] = s / BASE^(2j/E), j<E/2
    j = np.arange(HALF, dtype=np.float64)[:, None]
    pos = np.arange(S, dtype=np.float64)[None, :]
    theta = pos / np.power(np.float64(BASE_THETA), 2.0 * j / np.float64(E))
    return (np.cos(theta).astype(np.float16),
            np.sin(theta).astype(np.float16))


def _own_rows(r):
    # core r owns 128-row blocks {8j + r : j=0..3}
    return np.concatenate(
        [np.arange(128 * (8 * j + r), 128 * (8 * j + r) + 128) for j in range(4)])


def _prep_in_maps(inputs):
    f16 = np.float16
    x = np.ascontiguousarray(np.asarray(inputs["x"], dtype=np.float32))
    sf = np.asarray(inputs["scaling_factor"], dtype=np.float32)[:, None]
    wq_s = (sf * np.asarray(inputs["w_q"], dtype=np.float32)).astype(f16)
    wk_s = (sf * np.asarray(inputs["w_k"], dtype=np.float32)).astype(f16)
    wv_s = (sf * np.asarray(inputs["w_v"], dtype=np.float32)).astype(f16)
    wo_s = np.asarray(inputs["w_out"], dtype=np.float32).astype(f16)

    perm = _feature_perm()
    wq_p = np.ascontiguousarray(wq_s[:, perm])
    wk_p = np.ascontiguousarray(wk_s[:, perm])
    cosT, sinT = _rope_tables()

    col = np.arange(S)[None, :]
    in_maps = []
    for r in range(NC):
        rows = _own_rows(r)
        # mask for the two diagonal-candidate chunks of each row block:
        # block m4 (global block 8*m4+r) x key chunks {2*m4, 2*m4+1}
        am = np.zeros((RB, 2 * CH), dtype=f16)
        for m4 in range(4):
            rblk = rows[m4 * P:(m4 + 1) * P][:, None]
            cols = np.arange(2 * m4 * CH, (2 * m4 + 2) * CH)[None, :]
            am[m4 * P:(m4 + 1) * P] = np.where(
                cols > rblk, f16(-np.inf), f16(0.0))
        jsh = np.arange(r * 256, (r + 1) * 256)  # this core's rope rows (K)
        # K compute sees keys in gathered-chunk order (chunk n = core n's
        # scattered rows) -> K rope tables follow that column order
        pcols = np.concatenate([_own_rows(n) for n in range(NC)])
        in_maps.append({
            "x_r": np.ascontiguousarray(x[rows, :]),
            "wq": wq_p,
            "wk_c": np.ascontiguousarray(wk_p[:, r * RB:(r + 1) * RB]),
            "wv_c": np.ascontiguousarray(wv_s[:, r * RB:(r + 1) * RB]),
            "wo": wo_s,
            "cosq": np.ascontiguousarray(cosT[:, rows]),
            "sinq": np.ascontiguousarray(sinT[:, rows]),
            "cosk": np.ascontiguousarray(cosT[jsh][:, pcols]),
            "sink": np.ascontiguousarray(sinT[jsh][:, pcols]),
            "amask": am,
        })
    return in_maps


def _run(inputs, trace=False, **kw):
    global _BUILT
    from concourse.bass_utils import run_bass_kernel_spmd
    if _BUILT is None:
        _BUILT = _build_nc()
    in_maps = _prep_in_maps(inputs)
    res = run_bass_kernel_spmd(_BUILT, in_maps, list(range(NC)), trace=trace, **kw)
    out = np.empty((S, E), dtype=np.float16)
    for r in range(NC):
        out[_own_rows(r)] = np.asarray(res.results[r]["out_r"]).astype(np.float16)
    return out, res


def kernel(**inputs):
    out, _ = _run(inputs, trace=False)
    return out


# revision 11
# speedup vs baseline: 1.2049x; 1.0540x over previous
"""Bass/Tile TP attention kernel for trn2, 8 NeuronCores.

Sequence-parallel attention tuned for a gapless PE stream (~226 ns per
512-wide fp16 matmul incl. hidden FWL weight loads) and for HBM traffic
(the phases are near the compute/memory ridge):

  - weights pre-scaled by scaling_factor and cast to fp16 on host
  - wq/wk columns host-permuted even/odd per 512-block so RoPE pairs sit
    in separate 128-row tiles -> rope is pure element-wise DVE work
  - phases: norm -> [xq AllGather || Q proj (deep wq prefetch)] ->
    merged K/V loop (K frontloaded, V lags 2 chunks; xq read ONCE) ->
    [kT AllGather || V tail] -> [v half-gathers || scores+softmax] ->
    attnV (wo prefetch) -> out proj
  - qT spilled to DRAM between Q and scores to free SBUF for 3-deep xq
    chunk buffering in the merged loop
  - softmax: 1/64 folded into exp scale, ln64 shift keeps unnormalized
    P/attn in fp16 range, 1/sum applied free at out-proj psum eviction
  - v_b kept in chunk order, gathered in 2 halves, k-index remapped in
    attnV so the second half is never on the critical path

core r owns query row blocks {128*(8j+r) : j=0..3} (causal balance).
"""

import numpy as np

S = 4096
E = 4096
NC = 8
RB = S // NC          # 512 rows per core
P = 128
KO = E // P           # 32 k-tiles
NCH = 8               # 512-wide chunks over S or E
CH = 512
HALF = E // 2
EPS = 1e-6
BASE_THETA = 10000.0
LOG64 = float(np.log(64.0))

_BUILT = None


def _build_nc():
    import concourse.bass as bass
    import concourse.mybir as mybir
    import concourse.tile as tile
    from concourse import bacc
    from concourse.masks import make_identity

    dt16 = mybir.dt.float16
    dt32 = mybir.dt.float32
    AX = mybir.AxisListType.X
    mult = mybir.AluOpType.mult
    addop = mybir.AluOpType.add
    subop = mybir.AluOpType.subtract
    maxop = mybir.AluOpType.max
    Copy = mybir.ActivationFunctionType.Copy
    Exp = mybir.ActivationFunctionType.Exp
    Square = mybir.ActivationFunctionType.Square

    nc = bacc.Bacc(
        "TRN2", target_bir_lowering=False, debug=False, num_devices=NC)

    # I/O (weights arrive fp16, pre-scaled, wq/wk col-permuted even/odd)
    x_r = nc.dram_tensor("x_r", [RB, E], dt32, kind="ExternalInput")
    wq = nc.dram_tensor("wq", [E, E], dt16, kind="ExternalInput")
    wk_c = nc.dram_tensor("wk_c", [E, RB], dt16, kind="ExternalInput")
    wv_c = nc.dram_tensor("wv_c", [E, RB], dt16, kind="ExternalInput")
    wo = nc.dram_tensor("wo", [E, E], dt16, kind="ExternalInput")
    cosq_d = nc.dram_tensor("cosq", [HALF, RB], dt16, kind="ExternalInput")
    sinq_d = nc.dram_tensor("sinq", [HALF, RB], dt16, kind="ExternalInput")
    cosk_d = nc.dram_tensor("cosk", [2 * P, S], dt16, kind="ExternalInput")
    sink_d = nc.dram_tensor("sink", [2 * P, S], dt16, kind="ExternalInput")
    amask = nc.dram_tensor("amask", [RB, 2 * CH], dt16, kind="ExternalInput")
    out_r = nc.dram_tensor("out_r", [RB, E], dt16, kind="ExternalOutput")

    # internal DRAM
    qT_d = nc.dram_tensor("qT_d", [E, RB], dt16)  # roped q^T spill
    xqT_b = nc.dram_tensor("xqT_b", [E, RB], dt16)
    xqT_all = nc.dram_tensor("xqT_all", [NC * E, RB], dt16, addr_space="Shared")
    kT_b = nc.dram_tensor("kT_b", [RB, S], dt16)
    kT_all = nc.dram_tensor("kT_all", [E, S], dt16, addr_space="Shared")
    v_b0 = nc.dram_tensor("v_b0", [S // 2, RB], dt16)
    v_b1 = nc.dram_tensor("v_b1", [S // 2, RB], dt16)
    v_all0 = nc.dram_tensor("v_all0", [NC * S // 2, RB], dt16, addr_space="Shared")
    v_all1 = nc.dram_tensor("v_all1", [NC * S // 2, RB], dt16, addr_space="Shared")
    RG = [list(range(NC))]

    with tile.TileContext(nc) as tc:
        with tc.tile_pool(name="const", bufs=1) as constp, \
             tc.tile_pool(name="psQ", bufs=4, space="PSUM") as psQ, \
             tc.tile_pool(name="pstr", bufs=4, space="PSUM") as pstr:

            ident = constp.tile([P, P], dt16, tag="ident")
            make_identity(nc, ident)
            rinv_all = constp.tile([P, 4], dt32, tag="rinv_all")

            with tc.tile_pool(name="xqTrp", bufs=1) as xqTrp:
                xqT_r = xqTrp.tile([P, KO, RB], dt16, tag="xqT_r")

                # ---- stage A: RMS norm of own rows + transpose ----
                with tc.tile_pool(name="normp", bufs=2) as normp, \
                     tc.tile_pool(name="nstat", bufs=2) as nstat:
                    for t in range(RB // P):
                        x_sb = normp.tile([P, E], dt32, tag="x")
                        nc.sync.dma_start(x_sb, x_r[t * P:(t + 1) * P, :])
                        sq = normp.tile([P, E], dt32, tag="sq")
                        ssum = nstat.tile([P, 1], dt32, tag="ssum")
                        nc.scalar.activation(sq, x_sb, Square, accum_out=ssum)
                        s2 = nstat.tile([P, 1], dt32, tag="s2")
                        nc.vector.tensor_scalar(s2, ssum, 1.0 / E, EPS, mult, addop)
                        s3 = nstat.tile([P, 1], dt32, tag="s3")
                        nc.scalar.sqrt(s3, s2)
                        rinv = nstat.tile([P, 1], dt32, tag="rinv")
                        nc.vector.reciprocal(rinv, s3)
                        xq_sb = normp.tile([P, E], dt16, tag="xq")
                        nc.vector.tensor_scalar_mul(xq_sb, x_sb, rinv[:, 0:1])
                        for c in range(KO):
                            pt = pstr.tile([P, P], dt16, tag="tr")
                            nc.tensor.transpose(pt, xq_sb[:, c * P:(c + 1) * P], ident)
                            nc.scalar.copy(xqT_r[:, c, t * P:(t + 1) * P], pt)
                        nc.sync.dma_start(
                            xqT_b[:, t * P:(t + 1) * P].rearrange(
                                "(ko p) s -> p ko s", p=P),
                            xqT_r[:, :, t * P:(t + 1) * P])

                nc.gpsimd.collective_compute(
                    "AllGather", mybir.AluOpType.bypass, replica_groups=RG,
                    ins=[xqT_b[:]], outs=[xqT_all[:]])

                # ---- stage QT: qT = wq^T @ xq^T for own rows, + rope ----
                # deep wq prefetch (4 groups = 16MB) rides ahead of the
                # gather's HBM traffic; roped q^T spills to DRAM
                with tc.tile_pool(name="wqs", bufs=4) as wqs, \
                     tc.tile_pool(name="qring", bufs=8) as qring, \
                     tc.tile_pool(name="qropes", bufs=2) as qropes, \
                     tc.tile_pool(name="qrtmp", bufs=2) as qrtmp:
                    for mg in range(8):
                        wqt = wqs.tile([P, KO, CH], dt16, tag="wqt")
                        nc.sync.dma_start(
                            wqt, wq[:, mg * CH:(mg + 1) * CH].rearrange(
                                "(ko p) m -> p ko m", p=P))
                        qg = [qring.tile([P, RB], dt16, tag="qg", name=f"qg{i}")
                              for i in range(4)]
                        for m4 in range(4):
                            ps = psQ.tile([P, CH], dt32, tag="mm")
                            for k in range(KO):
                                nc.tensor.matmul(
                                    ps, lhsT=wqt[:, k, m4 * P:(m4 + 1) * P],
                                    rhs=xqT_r[:, k, :],
                                    start=(k == 0), stop=(k == KO - 1))
                            nc.scalar.copy(qg[m4], ps)
                        # rope pairs (h, 2+h) within this 512-col block
                        for h in range(2):
                            j0 = mg * 2 + h  # 128-row block into cosq/sinq
                            cq = qropes.tile([P, RB], dt16, tag="cq")
                            nc.sync.dma_start(cq, cosq_d[j0 * P:(j0 + 1) * P, :])
                            sq_ = qropes.tile([P, RB], dt16, tag="sq")
                            nc.sync.dma_start(sq_, sinq_d[j0 * P:(j0 + 1) * P, :])
                            t1 = qrtmp.tile([P, RB], dt16, tag="t1")
                            nc.vector.tensor_tensor(t1, qg[h], cq, mult)
                            t3 = qrtmp.tile([P, RB], dt16, tag="t3")
                            nc.vector.tensor_tensor(t3, qg[h], sq_, mult)
                            t4 = qrtmp.tile([P, RB], dt16, tag="t4")
                            nc.vector.tensor_tensor(t4, qg[2 + h], sq_, mult)
                            qE = qring.tile([P, RB], dt16, tag="qro", name="qE")
                            nc.vector.tensor_tensor(qE, t1, t4, addop)
                            t5 = qrtmp.tile([P, RB], dt16, tag="t5")
                            nc.vector.tensor_tensor(t5, qg[2 + h], cq, mult)
                            qO = qring.tile([P, RB], dt16, tag="qro", name="qO")
                            nc.vector.tensor_tensor(qO, t5, t3, subop)
                            me, mo = mg * 4 + h, mg * 4 + 2 + h
                            nc.sync.dma_start(qT_d[me * P:(me + 1) * P, :], qE)
                            nc.sync.dma_start(qT_d[mo * P:(mo + 1) * P, :], qO)

            # xqT_r freed
            xqa = xqT_all[:].rearrange("(c ko p) s -> c p ko s", c=NC, p=P)
            with tc.tile_pool(name="wkvp", bufs=1) as wkvp, \
                 tc.tile_pool(name="xqs", bufs=2) as xqs, \
                 tc.tile_pool(name="kevs", bufs=8) as kevs, \
                 tc.tile_pool(name="ktabs", bufs=2) as ktabs, \
                 tc.tile_pool(name="krtmp", bufs=2) as krtmp:
                wk16 = wkvp.tile([P, KO, RB], dt16, tag="wk16")
                wv16 = wkvp.tile([P, KO, RB], dt16, tag="wv16")
                for kc in range(4):
                    nc.sync.dma_start(
                        wk16[:, kc * 8:(kc + 1) * 8, :],
                        wk_c[kc * 8 * P:(kc + 1) * 8 * P, :].rearrange(
                            "(ko p) m -> p ko m", p=P))
                    nc.sync.dma_start(
                        wv16[:, kc * 8:(kc + 1) * 8, :],
                        wv_c[kc * 8 * P:(kc + 1) * 8 * P, :].rearrange(
                            "(ko p) m -> p ko m", p=P))

                def k_chunk(n):
                    xqg = xqs.tile([P, KO, CH], dt16, tag="xqg")
                    nc.sync.dma_start(xqg, xqa[n, :, :, :])
                    kev = [kevs.tile([P, CH], dt16, tag="kev", name=f"kev{i}")
                           for i in range(4)]
                    for m4 in range(4):
                        ps = psQ.tile([P, CH], dt32, tag="mm")
                        for k in range(KO):
                            nc.tensor.matmul(
                                ps, lhsT=wk16[:, k, m4 * P:(m4 + 1) * P],
                                rhs=xqg[:, k, :],
                                start=(k == 0), stop=(k == KO - 1))
                        nc.scalar.copy(kev[m4], ps)
                    for h in range(2):
                        ck = ktabs.tile([P, CH], dt16, tag="ck")
                        nc.sync.dma_start(
                            ck, cosk_d[h * P:(h + 1) * P, n * CH:(n + 1) * CH])
                        sk = ktabs.tile([P, CH], dt16, tag="sk")
                        nc.sync.dma_start(
                            sk, sink_d[h * P:(h + 1) * P, n * CH:(n + 1) * CH])
                        t1 = krtmp.tile([P, CH], dt16, tag="t1")
                        nc.vector.tensor_tensor(t1, kev[h], ck, mult)
                        t3 = krtmp.tile([P, CH], dt16, tag="t3")
                        nc.vector.tensor_tensor(t3, kev[h], sk, mult)
                        t4 = krtmp.tile([P, CH], dt16, tag="t4")
                        nc.vector.tensor_tensor(t4, kev[2 + h], sk, mult)
                        kroE = kevs.tile([P, CH], dt16, tag="kro", name="kroE")
                        nc.vector.tensor_tensor(kroE, t1, t4, addop)
                        t5 = krtmp.tile([P, CH], dt16, tag="t5")
                        nc.vector.tensor_tensor(t5, kev[2 + h], ck, mult)
                        kroO = kevs.tile([P, CH], dt16, tag="kro", name="kroO")
                        nc.vector.tensor_tensor(kroO, t5, t3, subop)
                        for jj in range(4):
                            g = 8 * jj + n  # global 128-col block
                            nc.sync.dma_start(
                                kT_b[h * P:(h + 1) * P, g * P:(g + 1) * P],
                                kroE[:, jj * P:(jj + 1) * P])
                            nc.sync.dma_start(
                                kT_b[(2 + h) * P:(3 + h) * P, g * P:(g + 1) * P],
                                kroO[:, jj * P:(jj + 1) * P])

                def v_chunk(n):
                    xqg = xqs.tile([P, KO, CH], dt16, tag="xqg")
                    nc.sync.dma_start(xqg, xqa[n, :, :, :])
                    vdst = v_b0 if n < 4 else v_b1
                    nl = n % 4
                    for ms in range(4):
                        ps = psQ.tile([P, CH], dt32, tag="mm")
                        for k in range(KO):
                            nc.tensor.matmul(
                                ps, lhsT=xqg[:, k, ms * P:(ms + 1) * P],
                                rhs=wv16[:, k, :],
                                start=(k == 0), stop=(k == KO - 1))
                        vev = kevs.tile([P, CH], dt16, tag="vev")
                        nc.scalar.copy(vev, ps)
                        # chunk-order rows: local block 4*nl + ms
                        nc.sync.dma_start(
                            vdst[(4 * nl + ms) * P:(4 * nl + ms + 1) * P, :], vev)

                # K all chunks first -> kT gather hides under the V loop;
                # v gathered in chunk-order halves so attnV never waits
                for n in range(NCH):
                    k_chunk(n)
                nc.gpsimd.collective_compute(
                    "AllGather", mybir.AluOpType.bypass, replica_groups=RG,
                    ins=[kT_b[:]], outs=[kT_all[:]])
                for n in range(NCH):
                    v_chunk(n)
                    if n == 3:  # v_b0 complete (chunks 0..3)
                        nc.gpsimd.collective_compute(
                            "AllGather", mybir.AluOpType.bypass,
                            replica_groups=RG, ins=[v_b0[:]], outs=[v_all0[:]])
                nc.gpsimd.collective_compute(
                    "AllGather", mybir.AluOpType.bypass, replica_groups=RG,
                    ins=[v_b1[:]], outs=[v_all1[:]])

            with tc.tile_pool(name="PTp", bufs=1) as PTp:
                PT = PTp.tile([P, KO, RB], dt16, tag="PT")

                # ---- scores + softmax + P^T ----
                kta = kT_all[:].rearrange("(ko p) s -> p ko s", p=P)
                with tc.tile_pool(name="qTtp", bufs=1) as qTtp, \
                     tc.tile_pool(name="Pp", bufs=1) as Pp, \
                     tc.tile_pool(name="kts", bufs=2) as kts, \
                     tc.tile_pool(name="sstat", bufs=4) as sstat, \
                     tc.tile_pool(name="ams", bufs=2) as ams:
                    qTt = qTtp.tile([P, KO, RB], dt16, tag="qTt")
                    nc.sync.dma_start(
                        qTt, qT_d[:].rearrange("(ko p) s -> p ko s", p=P))
                    Ptile = Pp.tile([P, 4, S], dt16, tag="P")

                    def softmax(m):
                        L = (2 * m + 2) * CH  # causal prefix length
                        maxv = sstat.tile([P, 1], dt32, tag="mx")
                        nc.vector.tensor_reduce(
                            maxv, Ptile[:, m, :L], axis=AX, op=maxop)
                        negb = sstat.tile([P, 1], dt32, tag="nb")
                        nc.vector.tensor_scalar(
                            negb, maxv, -1.0 / 64.0, -LOG64, mult, addop)
                        ssum = sstat.tile([P, 1], dt32, tag="sm")
                        nc.scalar.activation(
                            Ptile[:, m, :L], Ptile[:, m, :L], Exp,
                            bias=negb[:, 0:1], scale=1.0 / 64.0,
                            accum_out=ssum)
                        nc.vector.reciprocal(rinv_all[:, m:m + 1], ssum)

                    def p_transpose(m):
                        for st in range(8 * (m + 1)):
                            pt = pstr.tile([P, P], dt16, tag="tr")
                            nc.tensor.transpose(
                                pt, Ptile[:, m, st * P:(st + 1) * P], ident)
                            nc.scalar.copy(PT[:, st, m * P:(m + 1) * P], pt)

                    for n in range(NCH):
                        allowed = [m4 for m4 in range(4) if n <= 2 * m4 + 1]
                        kt = kts.tile([P, KO, CH], dt16, tag="kt")
                        nc.sync.dma_start(kt, kta[:, :, n * CH:(n + 1) * CH])
                        for m4 in allowed:
                            ps = psQ.tile([P, CH], dt32, tag="mm")
                            for k in range(KO):
                                nc.tensor.matmul(
                                    ps, lhsT=qTt[:, k, m4 * P:(m4 + 1) * P],
                                    rhs=kt[:, k, :],
                                    start=(k == 0), stop=(k == KO - 1))
                            if n in (2 * m4, 2 * m4 + 1):
                                am = ams.tile([P, CH], dt16, tag="am")
                                nc.sync.dma_start(
                                    am, amask[m4 * P:(m4 + 1) * P,
                                              (n - 2 * m4) * CH:
                                              (n - 2 * m4 + 1) * CH])
                                nc.vector.tensor_tensor(
                                    Ptile[:, m4, n * CH:(n + 1) * CH],
                                    ps, am, addop)
                            else:
                                nc.scalar.copy(
                                    Ptile[:, m4, n * CH:(n + 1) * CH], ps)
                        if n == 1:
                            softmax(0)
                            p_transpose(0)
                        elif n == 3:
                            softmax(1)
                            p_transpose(1)
                        elif n == 5:
                            softmax(2)
                        elif n == 6:
                            p_transpose(2)
                        elif n == 7:
                            softmax(3)
                            p_transpose(3)

                with tc.tile_pool(name="attnTp", bufs=1) as attnTp:
                    attnT = attnTp.tile([P, KO, RB], dt16, tag="attnT")

                    va0 = v_all0[:].rearrange(
                        "(c l p) eo -> c p l eo", c=NC, p=P)
                    va1 = v_all1[:].rearrange(
                        "(c l p) eo -> c p l eo", c=NC, p=P)
                    # global key block k -> (half, local block) in chunk order
                    kmap = []
                    for k in range(KO):
                        n, jj = k % 8, k // 8
                        kmap.append((n < 4, 4 * (n % 4) + jj))
                    korder = ([k for k in range(KO) if kmap[k][0]] +
                              [k for k in range(KO) if not kmap[k][0]])

                    with tc.tile_pool(name="wos", bufs=2) as wos, \
                         tc.tile_pool(name="vs", bufs=4) as vs, \
                         tc.tile_pool(name="oev", bufs=4) as oev:
                        # ---- attn^T = V x P^T ----
                        for m in range(KO):
                            c, sub = m // 4, m % 4
                            vlo = vs.tile([P, KO // 2, P], dt16, tag="vlo")
                            nc.sync.dma_start(
                                vlo, va0[c, :, :, sub * P:(sub + 1) * P])
                            vhi = vs.tile([P, KO // 2, P], dt16, tag="vhi")
                            nc.sync.dma_start(
                                vhi, va1[c, :, :, sub * P:(sub + 1) * P])
                            ps = psQ.tile([P, CH], dt32, tag="mm")
                            for i, k in enumerate(korder):
                                lo, lb = kmap[k]
                                vt = vlo if lo else vhi
                                j0 = k // 8
                                nc.tensor.matmul(
                                    ps[:, j0 * P:], lhsT=vt[:, lb, :],
                                    rhs=PT[:, k, j0 * P:],
                                    start=(i == 0), stop=(i == KO - 1))
                            nc.scalar.copy(attnT[:, m, :], ps)

                        # ---- out = attn @ wo, rows stay ours; /sum here ----
                        for n in range(NCH):
                            wot = wos.tile([P, KO, CH], dt16, tag="wot")
                            nc.sync.dma_start(
                                wot, wo[:, n * CH:(n + 1) * CH].rearrange(
                                    "(ko p) m -> p ko m", p=P))
                            for mq in range(4):
                                ps = psQ.tile([P, CH], dt32, tag="mm")
                                for k in range(KO):
                                    nc.tensor.matmul(
                                        ps,
                                        lhsT=attnT[:, k, mq * P:(mq + 1) * P],
                                        rhs=wot[:, k, :],
                                        start=(k == 0), stop=(k == KO - 1))
                                ot = oev.tile([P, CH], dt16, tag="ot")
                                nc.scalar.activation(
                                    ot, ps, Copy,
                                    scale=rinv_all[:, mq:mq + 1])
                                nc.sync.dma_start(
                                    out_r[mq * P:(mq + 1) * P,
                                          n * CH:(n + 1) * CH], ot)

    nc.compile()
    return nc


def _feature_perm():
    # per 512-block: first 256 even global features, then 256 odd
    blocks = []
    for c in range(NC):
        j = np.arange(256) + c * 256
        blocks.append(2 * j)
        blocks.append(2 * j + 1)
    return np.concatenate(blocks)


def _rope_tables():
    # feature-major half tables: theta[j, s] = s / BASE^(2j/E), j<E/2
    j = np.arange(HALF, dtype=np.float64)[:, None]
    pos = np.arange(S, dtype=np.float64)[None, :]
    theta = pos / np.power(np.float64(BASE_THETA), 2.0 * j / np.float64(E))
    return (np.cos(theta).astype(np.float16),
            np.sin(theta).astype(np.float16))


def _own_rows(r):
    # core r owns 128-row blocks {8j + r : j=0..3}
    return np.concatenate(
        [np.arange(128 * (8 * j + r), 128 * (8 * j + r) + 128) for j in range(4)])


def _prep_in_maps(inputs):
    f16 = np.float16
    x = np.ascontiguousarray(np.asarray(inputs["x"], dtype=np.float32))
    sf = np.asarray(inputs["scaling_factor"], dtype=np.float32)[:, None]
    wq_s = (sf * np.asarray(inputs["w_q"], dtype=np.float32)).astype(f16)
    wk_s = (sf * np.asarray(inputs["w_k"], dtype=np.float32)).astype(f16)
    wv_s = (sf * np.asarray(inputs["w_v"], dtype=np.float32)).astype(f16)
    wo_s = np.asarray(inputs["w_out"], dtype=np.float32).astype(f16)

    perm = _feature_perm()
    wq_p = np.ascontiguousarray(wq_s[:, perm])
    wk_p = np.ascontiguousarray(wk_s[:, perm])
    cosT, sinT = _rope_tables()

    col = np.arange(S)[None, :]
    in_maps = []
    for r in range(NC):
        rows = _own_rows(r)
        am = np.zeros((RB, 2 * CH), dtype=f16)
        for m4 in range(4):
            rblk = rows[m4 * P:(m4 + 1) * P][:, None]
            cols = np.arange(2 * m4 * CH, (2 * m4 + 2) * CH)[None, :]
            am[m4 * P:(m4 + 1) * P] = np.where(
                cols > rblk, f16(-np.inf), f16(0.0))
        jsh = np.arange(r * 256, (r + 1) * 256)  # this core's rope rows (K)
        # K compute sees keys in gathered-chunk order (chunk n = core n's
        # scattered rows) -> K rope tables follow that column order
        pcols = np.concatenate([_own_rows(n) for n in range(NC)])
        in_maps.append({
            "x_r": np.ascontiguousarray(x[rows, :]),
            "wq": wq_p,
            "wk_c": np.ascontiguousarray(wk_p[:, r * RB:(r + 1) * RB]),
            "wv_c": np.ascontiguousarray(wv_s[:, r * RB:(r + 1) * RB]),
            "wo": wo_s,
            "cosq": np.ascontiguousarray(cosT[:, rows]),
            "sinq": np.ascontiguousarray(sinT[:, rows]),
            "cosk": np.ascontiguousarray(cosT[jsh][:, pcols]),
            "sink": np.ascontiguousarray(sinT[jsh][:, pcols]),
            "amask": am,
        })
    return in_maps


def _run(inputs, trace=False, **kw):
    global _BUILT
    from concourse.bass_utils import run_bass_kernel_spmd
    if _BUILT is None:
        _BUILT = _build_nc()
    in_maps = _prep_in_maps(inputs)
    res = run_bass_kernel_spmd(_BUILT, in_maps, list(range(NC)), trace=trace, **kw)
    out = np.empty((S, E), dtype=np.float16)
    for r in range(NC):
        out[_own_rows(r)] = np.asarray(res.results[r]["out_r"]).astype(np.float16)
    return out, res


def kernel(**inputs):
    out, _ = _run(inputs, trace=False)
    return out
